# revision 21
# baseline (speedup 1.0000x reference)
"""AncProbsLayer Trainium2 kernel (8 NeuronCores, data-parallel over batch).

out[b,l,h,0,z] = sum_d seq[b,l,0,0,d] * P[b,h,d,z]
P[b,h] = diag(1/sqrt_pi_h) V_h diag(exp(lam_h * tau[b,h])) V_h^T diag(sqrt_pi_h)

The GTR eigendecomposition (H=8 symmetric 20x20 eigh) and per-(b,h) 20x20
transition matrices P are computed on host (tiny, constant per reference);
the device does the memory-bound (B*L,20)x(20,160) batched matmul.

Device structure (per core: BS=16 batches = 4 quads of 4):
- 4-way PE row tiling: a quad's 4 batches sit at SBUF partitions
  {0,32,64,96}+[0,20); four K=20 matmuls with tile_position=(32j,0) run
  CONCURRENTLY in the PE array (the PE is clock-gated to 1.2 GHz in this
  environment, so single-stream matmul is 4x too slow).
- uint8 output: each output column (b,h,z) carries scale 255/colbound
  folded into the host weights; f32 PSUM results are cast (RNE,
  saturating) to uint8 and DMA'd at 1 byte/elem; host dequantizes.
  Quantization rel-err ~2e-3, far under the 2e-2 gate.
- PSUM->SBUF casts are the critical resource (f32 PSUM reads run at
  1 elem/lane/cycle on DVE@0.96GHz / ACT@1.2GHz only). Structure:
  units of 4 PSUM banks filled to 480/512 by 3 consecutive 4-way rounds;
  each unit is cast by BOTH engines at once on a bank split (DVE owns
  members {0,1} in its own 2-bank PSUM tiles, ACT owns members {2,3}),
  double-buffered, so the cast engines share no tile and never serialize
  against each other.
- inputs: w and seq are packed per quad-member into one DRAM stripe so a
  single DMA (2 per member: w+quad0, quads1-3) covers them; triggers are
  split between the Sync and Scalar queues to halve head serialization.
"""

import sys

sys.path.insert(0, "/opt/trn_rl_repo")
sys.path.insert(0, "/root/.axon_site")

import numpy as np


def _install_axon_hooks_shim():
    try:
        import antenv.axon_hooks  # noqa: F401

        return
    except ImportError:
        pass
    try:
        import types

        mod = types.ModuleType("antenv.axon_hooks")
        _h = [None]
        mod.set_axon_ntff_profile_hook = lambda h: _h.__setitem__(0, h)
        mod.get_axon_ntff_profile_hook = lambda: _h[0]
        sys.modules["antenv.axon_hooks"] = mod
        import antenv

        antenv.axon_hooks = mod
        try:
            from trn_agent_boot.trn_boot import _ntff_profile_via_ctypes

            mod.set_axon_ntff_profile_hook(
                _ntff_profile_via_ctypes("/opt/axon/libaxon_pjrt.so")
            )
        except Exception:
            pass
    except Exception:
        pass


_install_axon_hooks_shim()

B, L, H, D = 128, 2048, 8, 20
N_CORES = 8
BS = B // N_CORES  # batches per core
NQ = BS // 4  # quads per core
HZ = H * D  # 160 output cols per (b, l)
LT = L // 128  # l-tiles per batch (16)

# per-quad unit sizes in l-tiles (sum = LT): 4 big + 2 small
UNIT_LT = [3, 3, 3, 3, 2, 2]
UNIT_LT0 = [0, 3, 6, 9, 12, 14]
NU = len(UNIT_LT)  # units per quad
_NC = None
LAST_RESULTS = None
LAST_IN_MAPS = None


def _unit_list():
    """Global unit order: (quad, unit_idx, lt0, nlt, out_col_offset_bytes)."""
    units = []
    off = 0
    for q in range(NQ):
        for i in range(NU):
            nl = UNIT_LT[i]
            units.append((q, i, UNIT_LT0[i], nl, off))
            off += nl * 640
    return units, off


def _dma_groups(n_units):
    """Groups of consecutive units per output DMA: singles at head/tail."""
    groups = [[0], [1]]
    u = 2
    while u < n_units - 2:
        if u + 1 < n_units - 2:
            groups.append([u, u + 1])
            u += 2
        else:
            groups.append([u])
            u += 1
    groups += [[n_units - 2], [n_units - 1]]
    return groups


def _build_nc():
    import concourse.bacc as bacc
    import concourse.tile as tile
    import concourse.mybir as mybir

    f32 = mybir.dt.float32
    bf16 = mybir.dt.bfloat16
    u8 = mybir.dt.uint8
    nc = bacc.Bacc(None, target_bir_lowering=False)

    # stripe per member j (rows 20j..20j+19): [w all quads | seq quad0] and
    # [seq quads 1..3]
    WQ = NQ * HZ  # 640 w cols
    swA = nc.declare_dram_parameter("swA", [4 * D, WQ + L], bf16, isOutput=False)
    sB = nc.declare_dram_parameter("sB", [4 * D, (NQ - 1) * L], bf16, isOutput=False)
    units, out_cols = _unit_list()
    out = nc.declare_dram_parameter("out", [128, out_cols], u8, isOutput=True)

    groups = _dma_groups(len(units))

    with tile.TileContext(nc) as tc:
        with (
            tc.tile_pool(name="spool", bufs=1) as spool,
            tc.tile_pool(name="pp", bufs=1, space="PSUM") as pp,
            tc.tile_pool(name="opool", bufs=5) as opool,
        ):
            # one tile: cols [0:WQ) = w, cols [WQ + q*L + l] = seq
            stw = spool.tile([128, WQ + NQ * L], bf16)
            for j in range(4):
                eng = nc.sync if j % 2 == 0 else nc.scalar
                eng.dma_start(
                    stw[32 * j : 32 * j + D, 0 : WQ + L], swA[20 * j : 20 * j + D, :]
                )
            for j in range(4):
                nc.sync.dma_start(
                    stw[32 * j : 32 * j + D, WQ + L :], sB[20 * j : 20 * j + D, :]
                )
            # separate per-engine PSUM tiles: DVE casts members {0,1} from
            # pd*, ACT casts members {2,3} from pa* -> the two cast engines
            # share no tile, so Tile never serializes them against each other
            pd0 = pp.tile([128, 2, 512], f32)
            pd1 = pp.tile([128, 2, 512], f32)
            pa0 = pp.tile([128, 2, 512], f32)
            pa1 = pp.tile([128, 2, 512], f32)
            pd = [pd0, pd1]
            pa = [pa0, pa1]

            gi = 0  # group index
            u_in_g = 0
            ot = None
            got_sizes = None

            for ui, (q, i, lt0, nl, off) in enumerate(units):
                ptd = pd[ui % 2]
                pta = pa[ui % 2]
                for r in range(nl):
                    t = lt0 + r
                    for j in (2, 3, 0, 1):
                        pt = ptd if j < 2 else pta
                        nc.tensor.matmul(
                            pt[:, j % 2, r * HZ : (r + 1) * HZ],
                            stw[
                                32 * j : 32 * j + D,
                                WQ + q * L + t * 128 : WQ + q * L + (t + 1) * 128,
                            ],
                            stw[32 * j : 32 * j + D, q * HZ : (q + 1) * HZ],
                            start=True,
                            stop=True,
                            tile_position=(32 * j, 0),
                        )
                if u_in_g == 0:
                    got_sizes = [units[k][3] * 640 for k in groups[gi]]
                    ot = opool.tile([128, sum(got_sizes)], u8, tag="ob")
                base = sum(got_sizes[:u_in_g])
                w = nl * HZ  # used cols per bank (480 or 320)
                dv = ot[:, base : base + 2 * w].rearrange("p (a b) -> p a b", a=2)
                av = ot[:, base + 2 * w : base + 4 * w].rearrange(
                    "p (a b) -> p a b", a=2
                )
                nc.scalar.copy(av, pta[:, :, 0:w])
                nc.vector.tensor_copy(dv, ptd[:, :, 0:w])
                u_in_g += 1
                if u_in_g == len(groups[gi]):
                    first_off = units[groups[gi][0]][4]
                    nbytes = sum(got_sizes)
                    nc.sync.dma_start(
                        out[:, first_off : first_off + nbytes], ot[:]
                    )
                    gi += 1
                    u_in_g = 0
    nc.compile()
    return nc


def _get_nc():
    global _NC
    if _NC is None:
        _NC = _build_nc()
    return _NC


def _host_precompute(rate_indices, tau_kernel, exchangeability_kernel, equilibrium_kernel):
    """Transition matrices P and uint8 column scales, in float64."""
    ek = exchangeability_kernel.astype(np.float64)[:, 0]
    eq = equilibrium_kernel.astype(np.float64)[:, 0]

    R = np.logaddexp(ek, 0.0)
    R = 0.5 * (R + R.transpose(0, 2, 1))
    m = eq.max(axis=-1, keepdims=True)
    p = np.exp(eq - m)
    p /= p.sum(axis=-1, keepdims=True)

    Q = R * p[:, None, :]
    diag = Q.sum(axis=-1)
    Q = Q - diag[:, :, None] * np.eye(D)
    mue = (p * diag).sum(axis=-1)
    Q = Q / np.maximum(mue, 1e-16)[:, None, None]

    sq = np.sqrt(p)
    isq = 1.0 / sq
    S = sq[:, :, None] * Q * isq[:, None, :]
    S = 0.5 * (S + S.transpose(0, 2, 1))
    lam, V = np.linalg.eigh(S)

    W1 = isq[:, :, None] * V
    W2 = V.transpose(0, 2, 1) * sq[:, None, :]

    tau_g = tau_kernel[rate_indices, np.arange(H)[None, :], 0].astype(np.float64)
    tau = np.logaddexp(np.clip(tau_g, -80.0, 80.0), 0.0)
    e = np.exp(lam[None, :, :] * tau[:, :, None])

    P = np.einsum("hdk,bhk,hkz->bhdz", W1, e, W2)  # (B, H, D, D)

    colbound = np.maximum(np.clip(P, 0, None).sum(axis=2), 1e-6)  # (B, H, Z)
    s = 255.0 / colbound
    Ps = P * s[:, :, None, :]
    wf = np.ascontiguousarray(Ps.transpose(0, 2, 1, 3)).reshape(B, D, HZ)
    scale = (colbound / 255.0).reshape(B, HZ).astype(np.float32)
    return wf.astype(np.float32), scale


def kernel(sequences, rate_indices, tau_kernel, exchangeability_kernel, equilibrium_kernel):
    global LAST_RESULTS, LAST_IN_MAPS
    from concourse.bass_utils import run_bass_kernel_spmd
    import ml_dtypes

    sequences = np.asarray(sequences)
    rate_indices = np.asarray(rate_indices)
    tau_kernel = np.asarray(tau_kernel)
    exchangeability_kernel = np.asarray(exchangeability_kernel)
    equilibrium_kernel = np.asarray(equilibrium_kernel)

    wf, scale = _host_precompute(
        rate_indices, tau_kernel, exchangeability_kernel, equilibrium_kernel
    )
    seq = np.asarray(sequences, dtype=np.float32).reshape(B, L, D)
    WQ = NQ * HZ

    in_maps = []
    for c in range(N_CORES):
        swA = np.empty((4 * D, WQ + L), dtype=ml_dtypes.bfloat16)
        sB = np.empty((4 * D, (NQ - 1) * L), dtype=ml_dtypes.bfloat16)
        for j in range(4):
            for q in range(NQ):
                b = c * BS + q * 4 + j
                swA[20 * j : 20 * j + D, q * HZ : (q + 1) * HZ] = wf[b]
                if q == 0:
                    swA[20 * j : 20 * j + D, WQ:] = seq[b].T
                else:
                    sB[20 * j : 20 * j + D, (q - 1) * L : q * L] = seq[b].T
        in_maps.append({"swA": swA, "sB": sB})

    LAST_IN_MAPS = in_maps
    nc = _get_nc()
    res = run_bass_kernel_spmd(nc, in_maps, core_ids=list(range(N_CORES)))
    LAST_RESULTS = res

    units, out_cols = _unit_list()
    outs = []
    for c in range(N_CORES):
        a = res.results[c]["out"]  # (128, out_cols) u8
        oc = np.empty((BS, L, HZ), dtype=np.float32)
        for (q, i, lt0, nl, off) in units:
            blk = a[:, off : off + nl * 640].reshape(128, 4, nl, HZ)
            # [p, j, li, hz] -> batch q*4+j, l = (lt0+li)*128 + p
            oc[
                q * 4 : q * 4 + 4, lt0 * 128 : (lt0 + nl) * 128, :
            ] = blk.transpose(1, 2, 0, 3).reshape(4, nl * 128, HZ)
        oc *= scale[c * BS : (c + 1) * BS, None, :]
        outs.append(oc)
    out = np.concatenate(outs, axis=0)
    return np.ascontiguousarray(out.reshape(B, L, H, 1, D))


# revision 22
# speedup vs baseline: 1.0148x; 1.0148x over previous
"""AncProbsLayer Trainium2 kernel (8 NeuronCores, data-parallel over batch).

out[b,l,h,0,z] = sum_d seq[b,l,0,0,d] * P[b,h,d,z]
P[b,h] = diag(1/sqrt_pi_h) V_h diag(exp(lam_h * tau[b,h])) V_h^T diag(sqrt_pi_h)

The GTR eigendecomposition (H=8 symmetric 20x20 eigh) and per-(b,h) 20x20
transition matrices P are computed on host (tiny, constant per reference);
the device does the memory-bound (B*L,20)x(20,160) batched matmul.

Device structure (per core: BS=16 batches = 4 quads of 4):
- 4-way PE row tiling: a quad's 4 batches sit at SBUF partitions
  {0,32,64,96}+[0,20); four K=20 matmuls with tile_position=(32j,0) run
  CONCURRENTLY in the PE array (the PE is clock-gated to 1.2 GHz in this
  environment, so single-stream matmul is 4x too slow).
- uint8 output: each output column (b,h,z) carries scale 255/colbound
  folded into the host weights; f32 PSUM results are cast (RNE,
  saturating) to uint8 and DMA'd at 1 byte/elem; host dequantizes.
  Quantization rel-err ~2e-3, far under the 2e-2 gate.
- PSUM->SBUF casts are the critical resource (f32 PSUM reads run at
  1 elem/lane/cycle on DVE@0.96GHz / ACT@1.2GHz only). Structure:
  units of 4 PSUM banks filled to 480/512 by 3 consecutive 4-way rounds;
  each unit is cast by BOTH engines at once on a bank split (DVE owns
  members {0,1} in its own 2-bank PSUM tiles, ACT owns members {2,3}),
  double-buffered, so the cast engines share no tile and never serialize
  against each other.
- inputs: w and seq are packed per quad-member into one DRAM stripe so a
  single DMA (2 per member: w+quad0, quads1-3) covers them; triggers are
  split between the Sync and Scalar queues to halve head serialization.
"""

import sys

sys.path.insert(0, "/opt/trn_rl_repo")
sys.path.insert(0, "/root/.axon_site")

import numpy as np


def _install_axon_hooks_shim():
    try:
        import antenv.axon_hooks  # noqa: F401

        return
    except ImportError:
        pass
    try:
        import types

        mod = types.ModuleType("antenv.axon_hooks")
        _h = [None]
        mod.set_axon_ntff_profile_hook = lambda h: _h.__setitem__(0, h)
        mod.get_axon_ntff_profile_hook = lambda: _h[0]
        sys.modules["antenv.axon_hooks"] = mod
        import antenv

        antenv.axon_hooks = mod
        try:
            from trn_agent_boot.trn_boot import _ntff_profile_via_ctypes

            mod.set_axon_ntff_profile_hook(
                _ntff_profile_via_ctypes("/opt/axon/libaxon_pjrt.so")
            )
        except Exception:
            pass
    except Exception:
        pass


_install_axon_hooks_shim()

B, L, H, D = 128, 2048, 8, 20
N_CORES = 8
BS = B // N_CORES  # batches per core
NQ = BS // 4  # quads per core
HZ = H * D  # 160 output cols per (b, l)
LT = L // 128  # l-tiles per batch (16)

# per-quad unit sizes in l-tiles (sum = LT): 4 big + 2 small
UNIT_LT = [3, 3, 3, 3, 2, 2]
UNIT_LT0 = [0, 3, 6, 9, 12, 14]
NU = len(UNIT_LT)  # units per quad
_NC = None
LAST_RESULTS = None
LAST_IN_MAPS = None


def _unit_list():
    """Global unit order: (quad, unit_idx, lt0, nlt, out_col_offset_bytes)."""
    units = []
    off = 0
    for q in range(NQ):
        for i in range(NU):
            nl = UNIT_LT[i]
            units.append((q, i, UNIT_LT0[i], nl, off))
            off += nl * 640
    return units, off


def _dma_groups(n_units):
    """Groups of consecutive units per output DMA: singles at head/tail."""
    groups = [[0], [1]]
    u = 2
    while u < n_units - 2:
        if u + 1 < n_units - 2:
            groups.append([u, u + 1])
            u += 2
        else:
            groups.append([u])
            u += 1
    groups += [[n_units - 2], [n_units - 1]]
    return groups


def _build_nc():
    import concourse.bacc as bacc
    import concourse.tile as tile
    import concourse.mybir as mybir

    f32 = mybir.dt.float32
    bf16 = mybir.dt.bfloat16
    u8 = mybir.dt.uint8
    nc = bacc.Bacc(None, target_bir_lowering=False)

    # stripe per member j (rows 20j..20j+19): [w all quads | seq quad0] and
    # [seq quads 1..3]
    WQ = NQ * HZ  # 640 w cols
    swA = nc.declare_dram_parameter("swA", [4 * D, WQ + L], bf16, isOutput=False)
    sB = nc.declare_dram_parameter("sB", [4 * D, (NQ - 1) * L], bf16, isOutput=False)
    units, out_cols = _unit_list()
    out = nc.declare_dram_parameter("out", [128, out_cols], u8, isOutput=True)

    groups = _dma_groups(len(units))

    with tile.TileContext(nc) as tc:
        with (
            tc.tile_pool(name="spool", bufs=1) as spool,
            tc.tile_pool(name="pp", bufs=1, space="PSUM") as pp,
            tc.tile_pool(name="opool", bufs=5) as opool,
        ):
            # one tile: cols [0:WQ) = w, cols [WQ + q*L + l] = seq
            stw = spool.tile([128, WQ + NQ * L], bf16)
            for j in range(4):
                eng = nc.sync if j % 2 == 0 else nc.scalar
                eng.dma_start(
                    stw[32 * j : 32 * j + D, 0 : WQ + L], swA[20 * j : 20 * j + D, :]
                )
            for j in range(4):
                nc.sync.dma_start(
                    stw[32 * j : 32 * j + D, WQ + L :], sB[20 * j : 20 * j + D, :]
                )
            # separate per-engine PSUM tiles: DVE casts members {0,1} from
            # pd*, ACT casts members {2,3} from pa* -> the two cast engines
            # share no tile, so Tile never serializes them against each other
            pd0 = pp.tile([128, 2, 512], f32)
            pd1 = pp.tile([128, 2, 512], f32)
            pa0 = pp.tile([128, 2, 512], f32)
            pa1 = pp.tile([128, 2, 512], f32)
            pd = [pd0, pd1]
            pa = [pa0, pa1]

            gi = 0  # group index
            u_in_g = 0
            ot = None
            got_sizes = None

            for ui, (q, i, lt0, nl, off) in enumerate(units):
                ptd = pd[ui % 2]
                pta = pa[ui % 2]
                for r in range(nl):
                    t = lt0 + r
                    for j in range(4):
                        pt = ptd if j < 2 else pta
                        nc.tensor.matmul(
                            pt[:, j % 2, r * HZ : (r + 1) * HZ],
                            stw[
                                32 * j : 32 * j + D,
                                WQ + q * L + t * 128 : WQ + q * L + (t + 1) * 128,
                            ],
                            stw[32 * j : 32 * j + D, q * HZ : (q + 1) * HZ],
                            start=True,
                            stop=True,
                            tile_position=(32 * j, 0),
                        )
                if u_in_g == 0:
                    got_sizes = [units[k][3] * 640 for k in groups[gi]]
                    ot = opool.tile([128, sum(got_sizes)], u8, tag="ob")
                base = sum(got_sizes[:u_in_g])
                w = nl * HZ  # used cols per bank (480 or 320)
                dv = ot[:, base : base + 2 * w].rearrange("p (a b) -> p a b", a=2)
                av = ot[:, base + 2 * w : base + 4 * w].rearrange(
                    "p (a b) -> p a b", a=2
                )
                nc.vector.tensor_copy(dv, ptd[:, :, 0:w])
                nc.scalar.copy(av, pta[:, :, 0:w])
                u_in_g += 1
                if u_in_g == len(groups[gi]):
                    first_off = units[groups[gi][0]][4]
                    nbytes = sum(got_sizes)
                    nc.sync.dma_start(
                        out[:, first_off : first_off + nbytes], ot[:]
                    )
                    gi += 1
                    u_in_g = 0
    nc.compile()
    return nc


def _get_nc():
    global _NC
    if _NC is None:
        _NC = _build_nc()
    return _NC


def _host_precompute(rate_indices, tau_kernel, exchangeability_kernel, equilibrium_kernel):
    """Transition matrices P and uint8 column scales, in float64."""
    ek = exchangeability_kernel.astype(np.float64)[:, 0]
    eq = equilibrium_kernel.astype(np.float64)[:, 0]

    R = np.logaddexp(ek, 0.0)
    R = 0.5 * (R + R.transpose(0, 2, 1))
    m = eq.max(axis=-1, keepdims=True)
    p = np.exp(eq - m)
    p /= p.sum(axis=-1, keepdims=True)

    Q = R * p[:, None, :]
    diag = Q.sum(axis=-1)
    Q = Q - diag[:, :, None] * np.eye(D)
    mue = (p * diag).sum(axis=-1)
    Q = Q / np.maximum(mue, 1e-16)[:, None, None]

    sq = np.sqrt(p)
    isq = 1.0 / sq
    S = sq[:, :, None] * Q * isq[:, None, :]
    S = 0.5 * (S + S.transpose(0, 2, 1))
    lam, V = np.linalg.eigh(S)

    W1 = isq[:, :, None] * V
    W2 = V.transpose(0, 2, 1) * sq[:, None, :]

    tau_g = tau_kernel[rate_indices, np.arange(H)[None, :], 0].astype(np.float64)
    tau = np.logaddexp(np.clip(tau_g, -80.0, 80.0), 0.0)
    e = np.exp(lam[None, :, :] * tau[:, :, None])

    P = np.einsum("hdk,bhk,hkz->bhdz", W1, e, W2)  # (B, H, D, D)

    colbound = np.maximum(np.clip(P, 0, None).sum(axis=2), 1e-6)  # (B, H, Z)
    s = 255.0 / colbound
    Ps = P * s[:, :, None, :]
    wf = np.ascontiguousarray(Ps.transpose(0, 2, 1, 3)).reshape(B, D, HZ)
    scale = (colbound / 255.0).reshape(B, HZ).astype(np.float32)
    return wf.astype(np.float32), scale


def kernel(sequences, rate_indices, tau_kernel, exchangeability_kernel, equilibrium_kernel):
    global LAST_RESULTS, LAST_IN_MAPS
    from concourse.bass_utils import run_bass_kernel_spmd
    import ml_dtypes

    sequences = np.asarray(sequences)
    rate_indices = np.asarray(rate_indices)
    tau_kernel = np.asarray(tau_kernel)
    exchangeability_kernel = np.asarray(exchangeability_kernel)
    equilibrium_kernel = np.asarray(equilibrium_kernel)

    wf, scale = _host_precompute(
        rate_indices, tau_kernel, exchangeability_kernel, equilibrium_kernel
    )
    seq = np.asarray(sequences, dtype=np.float32).reshape(B, L, D)
    WQ = NQ * HZ

    in_maps = []
    for c in range(N_CORES):
        swA = np.empty((4 * D, WQ + L), dtype=ml_dtypes.bfloat16)
        sB = np.empty((4 * D, (NQ - 1) * L), dtype=ml_dtypes.bfloat16)
        for j in range(4):
            for q in range(NQ):
                b = c * BS + q * 4 + j
                swA[20 * j : 20 * j + D, q * HZ : (q + 1) * HZ] = wf[b]
                if q == 0:
                    swA[20 * j : 20 * j + D, WQ:] = seq[b].T
                else:
                    sB[20 * j : 20 * j + D, (q - 1) * L : q * L] = seq[b].T
        in_maps.append({"swA": swA, "sB": sB})

    LAST_IN_MAPS = in_maps
    nc = _get_nc()
    res = run_bass_kernel_spmd(nc, in_maps, core_ids=list(range(N_CORES)))
    LAST_RESULTS = res

    units, out_cols = _unit_list()
    outs = []
    for c in range(N_CORES):
        a = res.results[c]["out"]  # (128, out_cols) u8
        oc = np.empty((BS, L, HZ), dtype=np.float32)
        for (q, i, lt0, nl, off) in units:
            blk = a[:, off : off + nl * 640].reshape(128, 4, nl, HZ)
            # [p, j, li, hz] -> batch q*4+j, l = (lt0+li)*128 + p
            oc[
                q * 4 : q * 4 + 4, lt0 * 128 : (lt0 + nl) * 128, :
            ] = blk.transpose(1, 2, 0, 3).reshape(4, nl * 128, HZ)
        oc *= scale[c * BS : (c + 1) * BS, None, :]
        outs.append(oc)
    out = np.concatenate(outs, axis=0)
    return np.ascontiguousarray(out.reshape(B, L, H, 1, D))
